# revision 1
# baseline (speedup 1.0000x reference)
"""Cross-attention kernel for TRN2, data-parallel over batch (B=8) on 8 cores.

Reference computation per batch element:
    xt  = proj_in(x)              # [L=4096, E=512], 1x1 conv == matmul
    Q   = xt @ W_q.T + b_q
    K   = ctx @ W_k.T + b_k       # ctx: [S=1024, E]
    V   = ctx @ W_v.T + b_v
    att = softmax(Q @ K.T * scale)
    out = proj_out((att @ V).T)   # [C=512, 64, 64]

Host-side algebraic folds (weights only, exact up to fp rounding):
  * scale, W_pi, W_q, W_k fold into G = (scale * W_q @ W_pi).T @ W_k, so
    logits.T = (G.T-contract ctx).T-contract X.  G is pre-scaled by 2^kg
    (weights-only bound) so its fp8 products stay in e4m3 normal range;
    the inverse scale rides the exp() activation's scale operand.
  * W_v and W_po fold:  WV = (W_po @ W_v).T ; b_o = b_po + W_po @ b_v

The two big attention GEMMs (logits ST = GC.T-c X and output U = VW.T-c PT)
run as fp8e4m3 DoubleRow matmuls: 256 contraction rows per instruction at
0.5 cycles/row -- 2x the bf16/fp32r MAC rate.  fp8's 3-bit mantissa cannot
represent softmax weights P ~= 1 +- 0.1 (quantization step 0.125 at 1.0),
so the softmax is mean-split:  P = 1 + p,  U = Vbar + sum_j p_j VW_j with
p = exp(s) - 1 cast to fp8 (full relative resolution on the deviation) and
Vbar = sum_j VW_j computed via a bf16 path (cbar = rowsum(ctx);
Vbar = cbar @ WV) so the dominant mean term carries no fp8 noise.
Z = 1024 + sum_j p_j from a DoubleRow ones-matmul over the same p8 tiles
(consistent with the numerator).

Per-core precomputes GC = G.T-c ctx and VW = ctx.T-c WV stay bf16 (fp8
operands there cost ~0.5e-2 extra max-norm error for only ~7us).

Measured end-to-end max-norm rel err of this quantization pipeline vs the
fp64 reference: ~1.1e-2 (budget 2e-2).
"""

import numpy as np
import ml_dtypes

import concourse.bass as bass
import concourse.mybir as mybir
import concourse.tile as tile
from concourse import bacc
from concourse.bass_utils import run_bass_kernel_spmd

F32 = mybir.dt.float32
BF16 = mybir.dt.bfloat16
F8 = mybir.dt.float8e4
EXP = mybir.ActivationFunctionType.Exp
DR = mybir.MatmulPerfMode.DoubleRow
AXX = mybir.AxisListType.X
ADD = mybir.AluOpType.add

NP_F8 = ml_dtypes.float8_e4m3
NP_BF = ml_dtypes.bfloat16

C = 512       # in channels
E = 512       # emb dim
L = 4096      # query length (64*64)
S = 1024      # key length (32*32)
LI = 512      # i-chunk (query) tile size
NCHUNK = L // LI
NCORES = 8

TRACE = False           # test harness can flip this before calling kernel()
LAST_RESULTS = None     # stashed BassKernelResults for the test harness

_PROGRAM_CACHE = {}


def _two(ap, inner):
    """[128, 2*inner] AP -> [128, 2, inner] for DoubleRow operands."""
    return ap.rearrange("p (two n) -> p two n", two=2, n=inner)


def _build_program(has_q0: bool, has_bo: bool, kg: int):
    nc = bacc.Bacc(
        "TRN2",
        target_bir_lowering=False,
        debug=False,
        enable_asserts=False,
        num_devices=NCORES,
    )
    x_d = nc.dram_tensor("x", [C, L], F8, kind="ExternalInput").ap()
    ctx_d = nc.dram_tensor("ctx", [E, S], BF16, kind="ExternalInput").ap()
    # gt arrives host-permuted into ct-major blocks: gt_d[p, ct*512+et*128+c']
    # = (2^kg * G.T)[et*128+p, ct*128+c'].
    gt_d = nc.dram_tensor("gt", [128, 4 * C], BF16, kind="ExternalInput").ap()
    wv_d = nc.dram_tensor("wv", [E, E], BF16, kind="ExternalInput").ap()
    # [128, 32] of ones; the Sigma-p DoubleRow lhsT reads cols {0, 16} so the
    # pair stride is 16B (dual-fp8 ldweights requires 16B-aligned even steps).
    one8_d = nc.dram_tensor("one8", [128, 32], F8, kind="ExternalInput").ap()
    one16_d = nc.dram_tensor("one16", [1, LI], BF16, kind="ExternalInput").ap()
    q0_d = bo_d = None
    if has_q0:
        q0_d = nc.dram_tensor("q0", [128, 8], F32, kind="ExternalInput").ap()
    if has_bo:
        bo_d = nc.dram_tensor("bo", [128, 4], F32, kind="ExternalInput").ap()
    y_d = nc.dram_tensor("y", [C, L], BF16, kind="ExternalOutput").ap()

    with tile.TileContext(nc) as tc:
        from contextlib import ExitStack

        with ExitStack() as ctx:
            cpool = ctx.enter_context(tc.tile_pool(name="consts", bufs=1))
            ps_s = ctx.enter_context(tc.tile_pool(name="ps_s", bufs=4, space="PSUM"))
            ps_z = ctx.enter_context(tc.tile_pool(name="ps_z", bufs=1, space="PSUM"))
            ps_u = ctx.enter_context(tc.tile_pool(name="ps_u", bufs=3, space="PSUM"))
            xpool = ctx.enter_context(tc.tile_pool(name="xp", bufs=2))
            ppool = ctx.enter_context(tc.tile_pool(name="pp", bufs=3))
            p8pool = ctx.enter_context(tc.tile_pool(name="p8p", bufs=2))
            opool = ctx.enter_context(tc.tile_pool(name="op", bufs=2))
            zpool = ctx.enter_context(tc.tile_pool(name="zp", bufs=2))

            # ---- loads in latency-priority order --------------------------
            one8_s = cpool.tile([128, 32], F8, name="one8s")
            nc.sync.dma_start(one8_s[:], one8_d[:, :])
            one16_s = cpool.tile([1, LI], BF16, name="one16s")
            nc.sync.dma_start(one16_s[:], one16_d[:, :])
            scale_imm = float(2.0 ** -kg)
            # gt ct=0 + ctx first halves unblock the jh=0 GC groups early.
            GTS = cpool.tile([128, 4 * C], BF16, name="gstk", tag="gstk")
            CTXT = cpool.tile([128, 4 * S], BF16, name="cstk", tag="cstk")
            nc.sync.dma_start(GTS[:, 0:512], gt_d[:, 0:512])
            for tt in range(4):
                nc.sync.dma_start(
                    CTXT[:, tt * S:tt * S + LI],
                    ctx_d[tt * 128:(tt + 1) * 128, 0:LI],
                )
            for ctb in range(1, 4):
                nc.sync.dma_start(
                    GTS[:, ctb * 512:(ctb + 1) * 512],
                    gt_d[:, ctb * 512:(ctb + 1) * 512],
                )

            def load_x(ic):
                xt = xpool.tile([128, 4 * LI], F8, name="xc", tag="x")
                nc.sync.dma_start(
                    xt[:].rearrange("p (t c) -> p t c", c=LI),
                    x_d[:, bass.ts(ic, LI)].rearrange("(t p) c -> p t c", p=128),
                )
                return xt

            X0 = load_x(0)                                        # prefetch chunk 0
            for tt in range(4):
                nc.sync.dma_start(
                    CTXT[:, tt * S + LI:(tt + 1) * S],
                    ctx_d[tt * 128:(tt + 1) * 128, LI:S],
                )
            WVT = cpool.tile([128, 4 * E], BF16, name="wstk", tag="wstk")
            for tt in range(4):
                nc.sync.dma_start(
                    WVT[:, tt * E:(tt + 1) * E],
                    wv_d[tt * 128:(tt + 1) * 128, :],
                )
            q0_s = bo_s = None
            if has_q0:
                q0_s = cpool.tile([128, 8], F32, name="q0s")
                nc.sync.dma_start(q0_s[:], q0_d[:, :])
            if has_bo:
                bo_s = cpool.tile([128, 4], F32, name="bos")
                nc.sync.dma_start(bo_s[:], bo_d[:, :])

            # ---- GC[c, j] = sum_e G[c, e] ctx[e, j]  (bf16, once) ---------
            # Output goes straight to the DoubleRow-interleaved fp8 layout:
            # GCD[cp][p, jt*256 + t*128 + m] = GC[(2cp+t)*128+p, jt*128+m].
            GCD = [
                cpool.tile([128, 2048], F8, name=f"gcd{cp}", tag=f"gcd{cp}")
                for cp in range(2)
            ]
            for jh in range(2):
                for ct in range(4):
                    gps = ps_s.tile([128, LI], F32, name="gps", tag="s")
                    for et in range(4):
                        nc.tensor.matmul(
                            gps[:],
                            GTS[:, ct * 512 + et * 128: ct * 512 + (et + 1) * 128],
                            CTXT[:, et * S + jh * LI: et * S + (jh + 1) * LI],
                            start=(et == 0),
                            stop=(et == 3),
                        )
                    dst = GCD[ct // 2][:, jh * 1024:(jh + 1) * 1024].rearrange(
                        "p (j two m) -> p j two m", two=2, m=128
                    )[:, :, ct % 2, :]
                    nc.vector.tensor_copy(
                        dst, gps[:].rearrange("p (j m) -> p j m", m=128)
                    )

            # ---- cbar[e] = sum_j ctx[e, j] ; Vbar = cbar @ WV (bf16) ------
            cbar = cpool.tile([128, 4], F32, name="cbar")
            nc.vector.tensor_reduce(
                cbar[:], CTXT[:].rearrange("p (t j) -> p t j", t=4), AXX, ADD
            )
            cbar16 = cpool.tile([128, 4], BF16, name="cbar16")
            nc.vector.tensor_copy(cbar16[:], cbar[:])

            def st_group(ic, jt, X, p8cur):
                """ST[j,i] for one j-tile: 2 DoubleRow fp8 matmuls, exp on
                scalar (with the 2^-kg descale), p8 = P - 1 cast on DVE."""
                sps = ps_s.tile([128, LI], F32, name="sps", tag="s")
                nc.tensor.matmul(
                    sps[:],
                    _two(GCD[0][:, jt * 256:(jt + 1) * 256], 128),
                    _two(X[:, 0:2 * LI], LI),
                    start=True,
                    stop=False,
                    perf_mode=DR,
                )
                nc.tensor.matmul(
                    sps[:],
                    _two(GCD[1][:, jt * 256:(jt + 1) * 256], 128),
                    _two(X[:, 2 * LI:4 * LI], LI),
                    start=False,
                    stop=True,
                    perf_mode=DR,
                )
                p = ppool.tile([128, LI], BF16, name="pt", tag=f"p{jt}")
                if has_q0:
                    nc.scalar.activation(
                        p[:], sps[:], EXP,
                        bias=q0_s[:, jt:jt + 1], scale=scale_imm,
                    )
                else:
                    nc.scalar.activation(p[:], sps[:], EXP, scale=scale_imm)
                jp, t = jt // 2, jt % 2
                if t == 0:
                    p8cur[jp] = p8pool.tile(
                        [128, 2 * LI], F8, name=f"p8_{jp}", tag=f"p8_{jp}"
                    )
                nc.vector.tensor_scalar_add(
                    p8cur[jp][:, t * LI:(t + 1) * LI], p[:], -1.0
                )

            def zsum_emit(p8prev):
                """Z - 1024 = sum_j p_j via DoubleRow ones-matmul, then the
                invz chain (DVE add/recip, gpsimd partition broadcast)."""
                zps = ps_z.tile([1, LI], F32, name="zps", tag="z")
                for jp in range(4):
                    nc.tensor.matmul(
                        zps[:],
                        _two(one8_s[:], 16)[:, :, 0:1],
                        _two(p8prev[jp][:], LI),
                        start=(jp == 0),
                        stop=(jp == 3),
                        perf_mode=DR,
                    )
                zr = zpool.tile([1, LI], F32, name="zr", tag="zr")
                nc.vector.tensor_scalar_add(zr[:], zps[:], 1024.0)
                invz = zpool.tile([1, LI], F32, name="invz", tag="invz")
                nc.vector.reciprocal_approx_fast(out=invz[:], in_=zr[:])
                invz_rep = zpool.tile([128, LI], F32, name="invzr", tag="invzr")
                nc.gpsimd.partition_broadcast(invz_rep[:], invz[:])
                return invz_rep

            def u_group(ic, ot, p8prev, VW8D, v16, invz_rep):
                """U[o,i] = Vbar[o] + sum_j p_j VW[j,o] (psum), then
                y = U * invz (DVE, bf16 out) and DMA out."""
                ups = ps_u.tile([128, LI], F32, name="ups", tag="u")
                nc.tensor.matmul(
                    ups[:],
                    v16[:, ot * 128:(ot + 1) * 128],
                    one16_s[:],
                    start=True,
                    stop=False,
                )
                for jp in range(4):
                    nc.tensor.matmul(
                        ups[:],
                        _two(VW8D[jp][:], E)[:, :, ot * 128:(ot + 1) * 128],
                        _two(p8prev[jp][:], LI),
                        start=False,
                        stop=(jp == 3),
                        perf_mode=DR,
                    )
                o = opool.tile([128, LI], BF16, name="ot", tag=f"o{ot}")
                nc.vector.tensor_mul(o[:], ups[:], invz_rep[:])
                if has_bo:
                    nc.vector.tensor_scalar_add(o[:], o[:], bo_s[:, ot:ot + 1])
                nc.sync.dma_start(y_d[ot * 128:(ot + 1) * 128, bass.ts(ic, LI)], o[:])

            # ---- window 0: ST(0), then VW + Vbar precompute ---------------
            X = X0
            Xnext = load_x(1)
            p8cur = {}
            st_group(0, 0, X, p8cur)
            st_group(0, 1, X, p8cur)
            # VW[j, o] = sum_e ctx[e, j] WV[e, o] (bf16, once), emitted after
            # the first ST groups so chunk 0's exp pipeline starts early; the
            # remaining ST groups interleave so sps psum slots recycle.
            VW8D = [None] * 4
            for jt in range(8):
                vps = ps_s.tile([128, E], F32, name="vps", tag="s")
                for et in range(4):
                    nc.tensor.matmul(
                        vps[:],
                        CTXT[:, et * S + jt * 128: et * S + (jt + 1) * 128],
                        WVT[:, et * E:(et + 1) * E],
                        start=(et == 0),
                        stop=(et == 3),
                    )
                jp, t = jt // 2, jt % 2
                if t == 0:
                    VW8D[jp] = cpool.tile(
                        [128, 2 * E], F8, name=f"vw8_{jp}", tag=f"vw8_{jp}"
                    )
                nc.vector.tensor_copy(VW8D[jp][:, t * E:(t + 1) * E], vps[:])
                if jt < 6:
                    st_group(0, jt + 2, X, p8cur)
            vb = ps_s.tile([1, E], F32, name="vb", tag="s")
            for et in range(4):
                nc.tensor.matmul(
                    vb[:],
                    cbar16[:, et:et + 1],
                    WVT[:, et * E:(et + 1) * E],
                    start=(et == 0),
                    stop=(et == 3),
                )
            v16 = cpool.tile([1, E], BF16, name="v16")
            nc.vector.tensor_copy(v16[:], vb[:])

            # ---- windows 1..8: ST(w) interleaved with U(w-1) --------------
            for w in range(1, NCHUNK + 1):
                p8prev, p8cur = p8cur, {}
                X, Xnext = Xnext, (load_x(w + 1) if w + 1 < NCHUNK else None)
                # Sigma-p(w-1) depends on the LAST cast of the previous chunk;
                # emitting the first ST pair ahead of it keeps the PE queue
                # fed with ready work at the window boundary (p-state ramp).
                invz_rep = None
                for k in range(4):
                    if w < NCHUNK:
                        st_group(w, 2 * k, X, p8cur)
                        st_group(w, 2 * k + 1, X, p8cur)
                    if k == 0:
                        invz_rep = zsum_emit(p8prev)
                    u_group(w - 1, k, p8prev, VW8D, v16, invz_rep)

    nc.compile()
    return nc


def kernel(**inputs) -> np.ndarray:
    global LAST_RESULTS
    x = np.asarray(inputs["x"], dtype=np.float32)
    context = np.asarray(inputs["context"], dtype=np.float32)
    W_pi = np.asarray(inputs["W_pi"], dtype=np.float64)
    b_pi = np.asarray(inputs["b_pi"], dtype=np.float64)
    W_q = np.asarray(inputs["W_q"], dtype=np.float64)
    b_q = np.asarray(inputs["b_q"], dtype=np.float64)
    W_k = np.asarray(inputs["W_k"], dtype=np.float64)
    W_v = np.asarray(inputs["W_v"], dtype=np.float64)
    b_v = np.asarray(inputs["b_v"], dtype=np.float64)
    W_po = np.asarray(inputs["W_po"], dtype=np.float64)
    b_po = np.asarray(inputs["b_po"], dtype=np.float64)

    scale = float(E) ** -0.5
    Wqpi = scale * (W_q @ W_pi)                            # [dq, c]
    G = (Wqpi.T @ W_k)                                     # [c, e]
    # fp8 pre-scale: |GC[c,j]| <= ||G[c,:]|| * ||ctx[:,j]|| and gaussian ctx
    # columns concentrate at sqrt(512)~22.6; 32 is a ~1.4x-margin bound.
    # Target max ~200 (e4m3 max normal is 240).
    rowg = float(np.linalg.norm(G, axis=1).max())
    kg = int(np.floor(np.log2(200.0 / (rowg * 32.0))))
    GT = np.ascontiguousarray(G.T * (2.0 ** kg)).astype(np.float32)   # [e, c]
    # ct-major block permutation: A[p, ct*512+et*128+c'] = GT[et*128+p, ct*128+c']
    GT = np.ascontiguousarray(
        GT.reshape(4, 128, 4, 128).transpose(1, 2, 0, 3).reshape(128, 4 * C)
    ).astype(NP_BF)
    b_row = scale * (W_q @ b_pi + b_q)
    q0_e = (W_k.T @ b_row).astype(np.float64)              # [e]
    WV = ((W_po @ W_v).T).astype(np.float32).astype(NP_BF)  # [e, o]
    b_o = (b_po + W_po @ b_v).astype(np.float32)           # [o]

    has_q0 = bool(np.any(q0_e))
    has_bo = bool(np.any(b_o))
    key = (has_q0, has_bo, kg)
    if key not in _PROGRAM_CACHE:
        _PROGRAM_CACHE[key] = _build_program(has_q0, has_bo, kg)
    nc = _PROGRAM_CACHE[key]

    one8 = np.ones((128, 32), dtype=NP_F8)
    one16 = np.ones((1, LI), dtype=NP_BF)
    in_maps = []
    for c in range(NCORES):
        ctx_mat = context[c].reshape(E, S)
        m = {
            "x": x[c].reshape(C, L).astype(NP_F8),
            "ctx": ctx_mat.astype(NP_BF),
            "gt": GT,
            "wv": WV,
            "one8": one8,
            "one16": one16,
        }
        if has_q0:
            # logits bias per key j: q0_e . ctx[:, j]  -> [S] -> [128, 8]
            q0j = (q0_e @ ctx_mat.astype(np.float64)).astype(np.float32)
            m["q0"] = np.ascontiguousarray(q0j.reshape(8, 128).T)
        if has_bo:
            m["bo"] = np.ascontiguousarray(b_o.reshape(4, 128).T)
        in_maps.append(m)

    res = run_bass_kernel_spmd(nc, in_maps, core_ids=list(range(NCORES)), trace=TRACE)
    LAST_RESULTS = res
    y = np.stack(
        [np.asarray(res.results[c]["y"]).astype(np.float32) for c in range(NCORES)],
        axis=0,
    )
    return np.ascontiguousarray(y.reshape(NCORES, C, 64, 64))



# revision 8
# speedup vs baseline: 1.0877x; 1.0877x over previous
"""Cross-attention kernel for TRN2, data-parallel over batch (B=8) on 8 cores.

Reference computation per batch element:
    xt  = proj_in(x)              # [L=4096, E=512], 1x1 conv == matmul
    Q   = xt @ W_q.T + b_q
    K   = ctx @ W_k.T + b_k       # ctx: [S=1024, E]
    V   = ctx @ W_v.T + b_v
    att = softmax(Q @ K.T * scale)
    out = proj_out((att @ V).T)   # [C=512, 64, 64]

Host-side algebraic folds (weights only, exact up to fp rounding):
  * scale, W_pi, W_q, W_k fold into G = (scale * W_q @ W_pi).T @ W_k, so
    logits.T = (G.T-contract ctx).T-contract X.
  * W_v and W_po fold:  WV = (W_po @ W_v).T ; b_o = b_po + W_po @ b_v
  * Vbar[o] = sum_j VW[j, o] = (ctx.sum over keys) @ WV  -- the softmax
    mean-numerator -- is a tiny per-core [512] vector, computed on host.

The two big attention GEMMs (logits ST = GC.T-c X and output U = VW.T-c PT)
run as fp8e4m3 DoubleRow matmuls: 256 contraction rows per instruction --
2x the bf16 MAC rate.  fp8's 3-bit mantissa cannot represent softmax
weights P ~= 1 +- 0.1 (quantization step 0.125 at 1.0), so the softmax is
mean-split:  P = 1 + p,  U = Vbar + sum_j p_j VW_j with p = exp(s) - 1
cast to fp8 (full relative resolution on the deviation).  The Vbar mean
term and the invz division both fold into one DVE scalar_tensor_tensor:
y = (U_dev + Vbar) * invz.

Z = 1024 + sum_j p_j comes from a DoubleRow ones-matmul whose lhsT has 128
ones columns, so the psum holds 128 identical Z rows -- the reciprocal is
then already partition-replicated and no gpsimd broadcast is needed.

The per-core precomputes GC = G.T-c ctx and VW = ctx.T-c WV also run as
fp8 DoubleRow matmuls (gt/ctx/wv arrive as scaled e4m3); the psum->fp8
casts run on the Scalar engine (idle during that phase) with the inverse
input scales folded into the activation scale.  Scales kg/kv are chosen
from exact host-side maxima so GCD/VW8 use the full e4m3 normal range.

A burst of tiny warm-up matmuls on a memset tile runs during the initial
DMA wait so the PE's HAM clock gate (cold = 1.2 GHz, warm = 2.4 GHz) is
already released when the first real matmul issues.
"""

import numpy as np
import ml_dtypes

import concourse.bass as bass
import concourse.mybir as mybir
import concourse.tile as tile
from concourse import bacc
from concourse.bass_utils import run_bass_kernel_spmd

F32 = mybir.dt.float32
BF16 = mybir.dt.bfloat16
F8 = mybir.dt.float8e4
EXP = mybir.ActivationFunctionType.Exp
IDENT = mybir.ActivationFunctionType.Identity
COPY = mybir.ActivationFunctionType.Copy
DR = mybir.MatmulPerfMode.DoubleRow
AXX = mybir.AxisListType.X
ADD = mybir.AluOpType.add
MUL = mybir.AluOpType.mult

NP_F8 = ml_dtypes.float8_e4m3
NP_BF = ml_dtypes.bfloat16

C = 512       # in channels
E = 512       # emb dim
L = 4096      # query length (64*64)
S = 1024      # key length (32*32)
LI = 512      # i-chunk (query) tile size
NCHUNK = L // LI
NCORES = 8
NWARM = 64    # PE warm-up matmuls during the DMA lead-in

TRACE = False           # test harness can flip this before calling kernel()
LAST_RESULTS = None     # stashed BassKernelResults for the test harness

_PROGRAM_CACHE = {}


def _two(ap, inner):
    """[128, 2*inner] AP -> [128, 2, inner] for DoubleRow operands."""
    return ap.rearrange("p (two n) -> p two n", two=2, n=inner)


def _build_program(has_q0: bool, has_bo: bool, kg: int, kc: int, kw: int, kv: int):
    nc = bacc.Bacc(
        "TRN2",
        target_bir_lowering=False,
        debug=False,
        enable_asserts=False,
        num_devices=NCORES,
    )
    x_d = nc.dram_tensor("x", [C, L], F8, kind="ExternalInput").ap()
    # ctx pre-scaled by 2^kc into e4m3 normal range
    ctx_d = nc.dram_tensor("ctx", [E, S], F8, kind="ExternalInput").ap()
    # gt arrives host-permuted into ct-major blocks: gt_d[p, ct*512+et*128+c']
    # = (2^kg * G.T)[et*128+p, ct*128+c'], e4m3.
    gt_d = nc.dram_tensor("gt", [128, 4 * C], F8, kind="ExternalInput").ap()
    # wv[p, et*E + o] = (2^kw * WV)[et*128+p, o], e4m3 (et-major blocks).
    wv_d = nc.dram_tensor("wv", [128, 4 * E], F8, kind="ExternalInput").ap()
    # ones for the Z (softmax denominator) DoubleRow matmul: 128 output rows
    # so the psum holds partition-replicated Z and feeds the DVE directly.
    one8_d = nc.dram_tensor("one8", [128, 256], F8, kind="ExternalInput").ap()
    # vbar_d[p, ot] = (2^kv * Vbar)[ot*128+p]
    vbar_d = nc.dram_tensor("vbar", [128, 4], F32, kind="ExternalInput").ap()
    q0_d = bo_d = None
    if has_q0:
        q0_d = nc.dram_tensor("q0", [128, 8], F32, kind="ExternalInput").ap()
    if has_bo:
        bo_d = nc.dram_tensor("bo", [128, 4], F32, kind="ExternalInput").ap()
    y_d = nc.dram_tensor("y", [C, L], BF16, kind="ExternalOutput").ap()

    exp_scale = float(2.0 ** -kg)
    gcd_scale = float(2.0 ** -kc)
    vw8_scale = float(2.0 ** (kv - kc - kw))
    zr_scale = float(2.0 ** kv)
    zr_bias = float(S) * zr_scale

    with tile.TileContext(nc) as tc:
        from contextlib import ExitStack

        with ExitStack() as ctx:
            cpool = ctx.enter_context(tc.tile_pool(name="consts", bufs=1))
            ps_s = ctx.enter_context(tc.tile_pool(name="ps_s", bufs=4, space="PSUM"))
            ps_z = ctx.enter_context(tc.tile_pool(name="ps_z", bufs=1, space="PSUM"))
            ps_u = ctx.enter_context(tc.tile_pool(name="ps_u", bufs=3, space="PSUM"))
            xpool = ctx.enter_context(tc.tile_pool(name="xp", bufs=2))
            ppool = ctx.enter_context(tc.tile_pool(name="pp", bufs=2))
            p8pool = ctx.enter_context(tc.tile_pool(name="p8p", bufs=2))
            opool = ctx.enter_context(tc.tile_pool(name="op", bufs=2))
            zpool = ctx.enter_context(tc.tile_pool(name="zp", bufs=2))

            # ---- PE warm-up: release the HAM clock gate during DMA wait ----
            warm = cpool.tile([128, 32], BF16, name="warm")
            nc.vector.memset(warm[:], 0.0)
            zrb = cpool.tile([128, 1], F32, name="zrb")
            nc.vector.memset(zrb[:], zr_bias)
            wps = ps_z.tile([32, 32], F32, name="wps", tag="z")
            for _ in range(NWARM):
                nc.tensor.matmul(wps[:], warm[:, 0:32], warm[:], start=True, stop=True)

            # ---- loads in latency-priority order --------------------------
            one8_s = cpool.tile([128, 256], F8, name="one8s")
            nc.sync.dma_start(one8_s[:], one8_d[:, :])
            GTS = cpool.tile([128, 4 * C], F8, name="gstk", tag="gstk")
            CTXT = cpool.tile([128, 4 * S], F8, name="cstk", tag="cstk")
            nc.sync.dma_start(GTS[:, 0:512], gt_d[:, 0:512])
            for tt in range(4):
                nc.sync.dma_start(
                    CTXT[:, tt * S:tt * S + LI],
                    ctx_d[tt * 128:(tt + 1) * 128, 0:LI],
                )
            for ctb in range(1, 4):
                nc.sync.dma_start(
                    GTS[:, ctb * 512:(ctb + 1) * 512],
                    gt_d[:, ctb * 512:(ctb + 1) * 512],
                )
            for tt in range(4):
                nc.sync.dma_start(
                    CTXT[:, tt * S + LI:(tt + 1) * S],
                    ctx_d[tt * 128:(tt + 1) * 128, LI:S],
                )
            WVT = cpool.tile([128, 4 * E], F8, name="wstk", tag="wstk")
            for tt in range(4):
                nc.sync.dma_start(
                    WVT[:, tt * E:(tt + 1) * E],
                    wv_d[:, tt * E:(tt + 1) * E],
                )

            def load_x(ic):
                xt = xpool.tile([128, 4 * LI], F8, name="xc", tag="x")
                nc.sync.dma_start(
                    xt[:].rearrange("p (t c) -> p t c", c=LI),
                    x_d[:, bass.ts(ic, LI)].rearrange("(t p) c -> p t c", p=128),
                )
                return xt

            X0 = load_x(0)                                        # prefetch chunk 0
            vbar_s = cpool.tile([128, 4], F32, name="vbars")
            nc.sync.dma_start(vbar_s[:], vbar_d[:, :])
            q0_s = bo_s = None
            if has_q0:
                q0_s = cpool.tile([128, 8], F32, name="q0s")
                nc.sync.dma_start(q0_s[:], q0_d[:, :])
            if has_bo:
                bo_s = cpool.tile([128, 4], F32, name="bos")
                nc.sync.dma_start(bo_s[:], bo_d[:, :])

            # ---- GC[c, j] = sum_e G[c, e] ctx[e, j]  (fp8 DR, once) -------
            # Output goes straight to the DoubleRow-interleaved fp8 layout:
            # GCD[cp][p, jt*256 + t*128 + m] = GC[(2cp+t)*128+p, jt*128+m],
            # scaled 2^kg (the 2^kc input scale divides out in the cast).
            GCD = [
                cpool.tile([128, 2048], F8, name=f"gcd{cp}", tag=f"gcd{cp}")
                for cp in range(2)
            ]
            for jh in range(2):
                for ct in range(4):
                    gps = ps_s.tile([128, LI], F32, name="gps", tag="s")
                    ctx4 = CTXT[:].rearrange("p (et j) -> p et j", et=4)
                    for eh in range(2):
                        nc.tensor.matmul(
                            gps[:],
                            _two(GTS[:, ct * 512 + eh * 256: ct * 512 + (eh + 1) * 256], 128),
                            ctx4[:, 2 * eh:2 * eh + 2, jh * LI:(jh + 1) * LI],
                            start=(eh == 0),
                            stop=(eh == 1),
                            perf_mode=DR,
                        )
                    dst = GCD[ct // 2][:, jh * 1024:(jh + 1) * 1024].rearrange(
                        "p (j two m) -> p j two m", two=2, m=128
                    )[:, :, ct % 2, :]
                    nc.scalar.activation(
                        dst, gps[:].rearrange("p (j m) -> p j m", m=128),
                        COPY, scale=gcd_scale,
                    )

            # ---- VW[j, o] = sum_e ctx[e, j] WV[e, o]  (fp8 DR, once) ------
            # VW8D[jp][p, t*E + o] = 2^kv * VW[(2jp+t)*128+p, o]
            VW8D = [None] * 4
            ctx4 = CTXT[:].rearrange("p (et j) -> p et j", et=4)
            for jt in range(8):
                vps = ps_s.tile([128, E], F32, name="vps", tag="s")
                for eh in range(2):
                    nc.tensor.matmul(
                        vps[:],
                        ctx4[:, 2 * eh:2 * eh + 2, jt * 128:(jt + 1) * 128],
                        _two(WVT[:, eh * 2 * E:(eh + 1) * 2 * E], E),
                        start=(eh == 0),
                        stop=(eh == 1),
                        perf_mode=DR,
                    )
                jp, t = jt // 2, jt % 2
                if t == 0:
                    VW8D[jp] = cpool.tile(
                        [128, 2 * E], F8, name=f"vw8_{jp}", tag=f"vw8_{jp}"
                    )
                nc.scalar.activation(
                    VW8D[jp][:, t * E:(t + 1) * E], vps[:], COPY, scale=vw8_scale
                )

            def st_group(ic, jt, X, pcur, p8cur):
                """ST[j,i] for one j-tile: 2 DoubleRow fp8 matmuls, exp on
                scalar (with the 2^-kg descale); after each jt-pair completes
                one DVE op casts p8 = P - 1 for the whole pair."""
                sps = ps_s.tile([128, LI], F32, name="sps", tag="s")
                nc.tensor.matmul(
                    sps[:],
                    _two(GCD[0][:, jt * 256:(jt + 1) * 256], 128),
                    _two(X[:, 0:2 * LI], LI),
                    start=True,
                    stop=False,
                    perf_mode=DR,
                )
                nc.tensor.matmul(
                    sps[:],
                    _two(GCD[1][:, jt * 256:(jt + 1) * 256], 128),
                    _two(X[:, 2 * LI:4 * LI], LI),
                    start=False,
                    stop=True,
                    perf_mode=DR,
                )
                jp, t = jt // 2, jt % 2
                if t == 0:
                    pcur[jp] = ppool.tile(
                        [128, 2 * LI], BF16, name=f"pt{jp}", tag=f"p{jp}"
                    )
                p = pcur[jp]
                if has_q0:
                    nc.scalar.activation(
                        p[:, t * LI:(t + 1) * LI], sps[:], EXP,
                        bias=q0_s[:, jt:jt + 1], scale=exp_scale,
                    )
                else:
                    nc.scalar.activation(
                        p[:, t * LI:(t + 1) * LI], sps[:], EXP, scale=exp_scale
                    )
                if t == 1:
                    p8cur[jp] = p8pool.tile(
                        [128, 2 * LI], F8, name=f"p8_{jp}", tag=f"p8_{jp}"
                    )
                    nc.vector.tensor_scalar_add(p8cur[jp][:], p[:], -1.0)

            def zsum_emit(p8prev):
                """2^kv * Z rows (all 128 partitions identical) via DoubleRow
                ones-matmul; zr = 2^kv*(1024 + sum p) on Scalar, reciprocal
                on DVE -- already partition-replicated, no broadcast."""
                zps = ps_z.tile([128, LI], F32, name="zps", tag="z")
                for jp in range(4):
                    nc.tensor.matmul(
                        zps[:],
                        _two(one8_s[:], 128),
                        _two(p8prev[jp][:], LI),
                        start=(jp == 0),
                        stop=(jp == 3),
                        perf_mode=DR,
                    )
                zr = zpool.tile([128, LI], F32, name="zr", tag="zr")
                nc.scalar.activation(zr[:], zps[:], IDENT, bias=zrb[:, 0:1], scale=zr_scale)
                invz = zpool.tile([128, LI], F32, name="invz", tag="invz")
                nc.vector.reciprocal_approx_fast(out=invz[:], in_=zr[:])
                return invz

            def u_group(ic, ot, p8prev, invz):
                """U_dev[o,i] = sum_j p_j VW[j,o] (psum, scaled 2^kv), then
                y = (U_dev + 2^kv Vbar) * invz on DVE (bf16 out) and DMA."""
                ups = ps_u.tile([128, LI], F32, name="ups", tag="u")
                for jp in range(4):
                    nc.tensor.matmul(
                        ups[:],
                        _two(VW8D[jp][:], E)[:, :, ot * 128:(ot + 1) * 128],
                        _two(p8prev[jp][:], LI),
                        start=(jp == 0),
                        stop=(jp == 3),
                        perf_mode=DR,
                    )
                o = opool.tile([128, LI], BF16, name="ot", tag=f"o{ot}")
                nc.vector.scalar_tensor_tensor(
                    o[:], ups[:], vbar_s[:, ot:ot + 1], invz[:], ADD, MUL
                )
                if has_bo:
                    nc.vector.tensor_scalar_add(o[:], o[:], bo_s[:, ot:ot + 1])
                nc.sync.dma_start(y_d[ot * 128:(ot + 1) * 128, bass.ts(ic, LI)], o[:])

            # ---- window 0: ST(0) after the GC/VW precompute ---------------
            X = X0
            Xnext = load_x(1)
            pcur, p8cur = {}, {}
            for jt in range(8):
                st_group(0, jt, X, pcur, p8cur)

            # ---- windows 1..8: ST(w) interleaved with U(w-1) --------------
            for w in range(1, NCHUNK + 1):
                p8prev, p8cur = p8cur, {}
                pcur = {}
                X, Xnext = Xnext, (load_x(w + 1) if w + 1 < NCHUNK else None)
                invz = None
                for k in range(4):
                    if w < NCHUNK:
                        st_group(w, 2 * k, X, pcur, p8cur)
                        st_group(w, 2 * k + 1, X, pcur, p8cur)
                    if k == 0:
                        invz = zsum_emit(p8prev)
                    u_group(w - 1, k, p8prev, invz)

    nc.compile()
    return nc


def kernel(**inputs) -> np.ndarray:
    global LAST_RESULTS
    x = np.asarray(inputs["x"], dtype=np.float32)
    context = np.asarray(inputs["context"], dtype=np.float32)
    W_pi = np.asarray(inputs["W_pi"], dtype=np.float64)
    b_pi = np.asarray(inputs["b_pi"], dtype=np.float64)
    W_q = np.asarray(inputs["W_q"], dtype=np.float64)
    b_q = np.asarray(inputs["b_q"], dtype=np.float64)
    W_k = np.asarray(inputs["W_k"], dtype=np.float64)
    W_v = np.asarray(inputs["W_v"], dtype=np.float64)
    b_v = np.asarray(inputs["b_v"], dtype=np.float64)
    W_po = np.asarray(inputs["W_po"], dtype=np.float64)
    b_po = np.asarray(inputs["b_po"], dtype=np.float64)

    scale = float(E) ** -0.5
    Wqpi = scale * (W_q @ W_pi)                            # [dq, c]
    G = (Wqpi.T @ W_k)                                     # [c, e]
    b_row = scale * (W_q @ b_pi + b_q)
    q0_e = (W_k.T @ b_row).astype(np.float64)              # [e]
    WV64 = (W_po @ W_v).T                                  # [e, o]
    b_o = (b_po + W_po @ b_v).astype(np.float32)           # [o]

    ctx_all = context.reshape(NCORES, E, S)
    G32 = G.astype(np.float32)
    # exact per-core maxima for the fp8 scale choices
    gc_max = 1e-30
    vw_max = 1e-30
    ctx_max = float(np.abs(ctx_all).max())
    WV32 = WV64.astype(np.float32)
    for c in range(NCORES):
        gc_max = max(gc_max, float(np.abs(G32 @ ctx_all[c]).max()))
        vw_max = max(vw_max, float(np.abs(ctx_all[c].T @ WV32).max()))
    kc = int(np.floor(np.log2(200.0 / ctx_max)))
    kw = int(np.floor(np.log2(200.0 / max(float(np.abs(WV64).max()), 1e-30))))
    kg = int(np.floor(np.log2(200.0 / gc_max)))
    kv = int(np.floor(np.log2(200.0 / vw_max)))

    # TRN e4m3 tops out at +-240 (S.1111.000 is inf), so clip before casting.
    GT = np.clip(G.T * (2.0 ** kg), -240.0, 240.0).astype(np.float32)   # [e, c]
    # ct-major block permutation: A[p, ct*512+et*128+c'] = GT[et*128+p, ct*128+c']
    GT = np.ascontiguousarray(
        GT.reshape(4, 128, 4, 128).transpose(1, 2, 0, 3).reshape(128, 4 * C)
    ).astype(NP_F8)
    # wv et-major blocks: [p, et*E + o] = 2^kw WV[et*128+p, o]
    WVS = np.ascontiguousarray(
        np.clip(WV64 * (2.0 ** kw), -240.0, 240.0).astype(np.float32)
        .reshape(4, 128, E).transpose(1, 0, 2).reshape(128, 4 * E)
    ).astype(NP_F8)

    has_q0 = bool(np.any(q0_e))
    has_bo = bool(np.any(b_o))
    key = (has_q0, has_bo, kg, kc, kw, kv)
    if key not in _PROGRAM_CACHE:
        _PROGRAM_CACHE[key] = _build_program(has_q0, has_bo, kg, kc, kw, kv)
    nc = _PROGRAM_CACHE[key]

    one8 = np.ones((128, 256), dtype=NP_F8)
    in_maps = []
    for c in range(NCORES):
        ctx_mat = ctx_all[c]
        vbar = (ctx_mat.sum(axis=1).astype(np.float64) @ WV64) * (2.0 ** kv)
        m = {
            "x": x[c].reshape(C, L).astype(NP_F8),
            "ctx": np.clip(ctx_mat * (2.0 ** kc), -240.0, 240.0).astype(NP_F8),
            "gt": GT,
            "wv": WVS,
            "one8": one8,
            "vbar": np.ascontiguousarray(
                vbar.astype(np.float32).reshape(4, 128).T
            ),
        }
        if has_q0:
            # logits bias per key j: q0_e . ctx[:, j]  -> [S] -> [128, 8]
            q0j = (q0_e @ ctx_mat.astype(np.float64)).astype(np.float32)
            m["q0"] = np.ascontiguousarray(q0j.reshape(8, 128).T)
        if has_bo:
            m["bo"] = np.ascontiguousarray(b_o.reshape(4, 128).T)
        in_maps.append(m)

    res = run_bass_kernel_spmd(nc, in_maps, core_ids=list(range(NCORES)), trace=TRACE)
    LAST_RESULTS = res
    y = np.stack(
        [np.asarray(res.results[c]["y"]).astype(np.float32) for c in range(NCORES)],
        axis=0,
    )
    return np.ascontiguousarray(y.reshape(NCORES, C, 64, 64))
